# revision 1
# baseline (speedup 1.0000x reference)
"""Cascade (multi-level paged) attention, distributed over 8 TRN2 NeuronCores.

Sharding: tensor-parallel over the 8 KV heads — core k owns kv-head k and its
4 GQA query heads for all 32 sequences.  Each core then reads exactly 1/8 of
the paged KV cache (its head's slice of the shared L0 prefix plus every
sequence's L1/L2 pages) from HBM once, which is the minimum possible traffic,
and no inter-core communication is needed.

Host-side prep (part of kernel(), done in numpy):
  * gather pages in the order [L0 | seq0 L1,L2 | seq1 L1,L2 | ...] using the
    page-index tensors,
  * K laid out d-major  [128 d, 53248 tok]  (matmul stationary operand),
  * V laid out token-major with a ones-column appended [tok, 129] and
    pre-swizzled into [128 tok-in-chunk, 416 chunk * 129] so each 128-token
    chunk is a direct SBUF slice; the ones column makes the softmax
    denominator fall out of the PV matmul's last output column,
  * q transposed to [128 d, 128 (seq,group)] per core.

Device kernel (per core), streaming 128-token chunks:
  scores^T chunk = matmul(lhsT=K_chunk [d,128tok], rhs=qT [d,nq]) -> PSUM
  probs = exp(scale * scores) via ScalarE (no max subtraction: scores are
  ~N(0,1) after scaling, exp is safe in f32, and partial attention sums
  become directly addable so the shared-L0 partial and per-sequence partial
  merge with a single add)
  out  += matmul(lhsT=probs^T chunk [tok,nq], rhs=[V|1] chunk [tok,129])
  epilogue per seq: (seq partial + L0 partial)[:, :128] * (1/[..., 128]).

Scores for many chunks are batched into one PSUM bank so one ACT exp call
covers up to 512 columns.  Banks are software-pipelined: PV of bank i is
emitted after the score matmuls of bank i+1 so the PE never waits on ACT.
The shared-L0 banks run first; their partial is bounced once through DRAM
into a [4 (g), seq*129] layout so each seq bank merges, divides and writes
its own output rows locally (DVE ops cannot address partition offsets that
are not multiples of 32, DRAM APs can). The final seq banks shrink to 1
sequence so the tail after the last KV byte lands is ~1.5 us.
"""

import os
from contextlib import ExitStack

import numpy as np
import ml_dtypes

import concourse.mybir as mybir
import concourse.tile as tile
from concourse import bacc
from concourse.bass_utils import run_bass_kernel_spmd

# ---- problem constants (hardcoded; kernel.py must be self-contained) ----
B = 32          # sequences
HKV = 8         # kv heads == number of cores
G = 4           # query heads per kv head
D = 128         # head dim
L0_T = 4096     # shared-prefix tokens
SEQ_T = 1536    # per-sequence tokens (L1 1024 + L2 512)
T_ALL = L0_T + B * SEQ_T        # 53248
CH = T_ALL // 128               # 416 chunks of 128 tokens
L0_CH = L0_T // 128             # 32
SEQ_CH = SEQ_T // 128           # 12
SCALE = 0.08838834764831845     # D ** -0.5
VW = D + 1                      # V width incl. ones column

# chunks per DMA tile. L0 (processed first) starts with a small tile so the
# PE can start early; seq-region tiles shrink at the end so the last-arriving
# data feeds only a tiny epilogue.
TILE_CHUNKS = [8, 24] + [48] * 7 + [24, 12, 12]
assert sum(TILE_CHUNKS) == CH
TILE_START = [sum(TILE_CHUNKS[:i]) for i in range(len(TILE_CHUNKS))]
CHUNK_TILE = []                 # chunk -> (tile idx, chunk offset within tile)
for t, n in enumerate(TILE_CHUNKS):
    for c in range(n):
        CHUNK_TILE.append((t, c))
# seq banks: (first seq, count); sized 4 for most, shrinking at the end
SEQ_BANKS = [(0, 4), (4, 4), (8, 4), (12, 4), (16, 4), (20, 4), (24, 4),
             (28, 2), (30, 1), (31, 1)]

F32 = mybir.dt.float32


def _dtype_cfg():
    name = os.environ.get("KERNEL_DTYPE", "bf16")
    if name == "f32":
        return mybir.dt.float32, np.float32
    return mybir.dt.bfloat16, ml_dtypes.bfloat16


def build_nc(dt):
    """Builds the single-core Bass/Tile graph (same graph runs SPMD on 8 cores)."""
    nc = bacc.Bacc("TRN2", target_bir_lowering=False, debug=False)
    k_ext = nc.declare_dram_parameter("k", [128, T_ALL], dt, isOutput=False)
    v_ext = nc.declare_dram_parameter("v", [128, CH * VW], dt, isOutput=False)
    q_ext = nc.declare_dram_parameter("qt", [128, B * G], dt, isOutput=False)
    out_ext = nc.declare_dram_parameter("out", [B * G, D], F32, isOutput=True)
    l0b_dram = nc.dram_tensor("bounce", [B * G * VW], F32)

    # bank schedule: a "bank" is one PSUM score tile [128, <=512].
    # L0 banks (4 chunks x 128 qcols) run first; their merged partial is
    # bounced through DRAM into a [4, B*VW] (partition=g, seq along free)
    # layout so every seq bank can merge + divide + write its own output rows
    # locally - the tail after the last DMA is one tiny epilogue.
    banks = [("l0", j, None) for j in range(L0_CH // 4)] \
        + [("seq", s0, n) for (s0, n) in SEQ_BANKS]

    kv_bufs = 5 if dt == mybir.dt.bfloat16 else 3
    with tile.TileContext(nc) as tc:
        with ExitStack() as ctx:
            kpool = ctx.enter_context(tc.tile_pool(name="kp", bufs=kv_bufs))
            vpool = ctx.enter_context(tc.tile_pool(name="vp", bufs=kv_bufs))
            qpool = ctx.enter_context(tc.tile_pool(name="qp", bufs=1))
            epool = ctx.enter_context(tc.tile_pool(name="ep", bufs=2))
            apool = ctx.enter_context(tc.tile_pool(name="ap", bufs=1))
            sspool = ctx.enter_context(tc.tile_pool(name="ssp", bufs=6))
            obpool = ctx.enter_context(tc.tile_pool(name="obp", bufs=5))
            rpool = ctx.enter_context(tc.tile_pool(name="rp", bufs=8))
            scpool = ctx.enter_context(tc.tile_pool(name="scp", bufs=3, space="PSUM"))
            l0pool = ctx.enter_context(tc.tile_pool(name="l0p", bufs=1, space="PSUM"))
            sapool = ctx.enter_context(tc.tile_pool(name="sap", bufs=4, space="PSUM"))

            qt = qpool.tile([128, B * G], dt, tag="qt")
            nc.sync.dma_start(qt[:], q_ext[:])

            l0acc = l0pool.tile([128, VW], F32, tag="l0acc")
            l0sb = apool.tile([128, VW], F32, tag="l0sb")
            # L0 partial rearranged to partition=g, seq along the free dim
            l0ss = apool.tile([4, B * VW], F32, tag="l0ss")

            ktiles, vtiles = {}, {}

            def kv(t):
                if t not in ktiles:
                    n, c0 = TILE_CHUNKS[t], TILE_START[t]
                    kt = kpool.tile([128, n * 128], dt, tag="kt")
                    vt = vpool.tile([128, n * VW], dt, tag="vt")
                    # split big tiles into half-DMAs so the PE's wait per
                    # score group stays under the ~3.4us HAM idle window
                    h = n // 2 if n >= 48 else n
                    for a in range(0, n, h):
                        b = min(a + h, n)
                        nc.sync.dma_start(
                            kt[:, a * 128:b * 128],
                            k_ext[:, (c0 + a) * 128:(c0 + b) * 128])
                        nc.sync.dma_start(
                            vt[:, a * VW:b * VW],
                            v_ext[:, (c0 + a) * VW:(c0 + b) * VW])
                    ktiles[t], vtiles[t] = kt, vt
                return ktiles[t], vtiles[t]

            def emit_scores(bank):
                kind, j, n = bank
                sc = scpool.tile([128, 512], F32, tag="sc")
                if kind == "l0":
                    for jl in range(4):
                        chunk = 4 * j + jl
                        t, coff = CHUNK_TILE[chunk]
                        kt, _ = kv(t)
                        nc.tensor.matmul(
                            out=sc[:, 128 * jl:128 * jl + 128],
                            lhsT=kt[:, coff * 128:coff * 128 + 128],
                            rhs=qt[:, 0:128],
                            start=True, stop=True,
                        )
                else:
                    for bl in range(n):
                        s = j + bl
                        for c in range(SEQ_CH):
                            chunk = L0_CH + s * SEQ_CH + c
                            t, coff = CHUNK_TILE[chunk]
                            kt, _ = kv(t)
                            col = 48 * bl + 4 * c
                            nc.tensor.matmul(
                                out=sc[:, col:col + 4],
                                lhsT=kt[:, coff * 128:coff * 128 + 128],
                                rhs=qt[:, 4 * s:4 * s + 4],
                                start=True, stop=True,
                            )
                return sc

            pending_out = []

            def emit_tail(bank, sc):
                kind, j, n = bank
                used = 512 if kind == "l0" else 48 * n
                et = epool.tile([128, 512], dt, tag="et")
                nc.scalar.activation(
                    et[:, :used], sc[:, :used],
                    mybir.ActivationFunctionType.Exp, scale=SCALE,
                )
                # flush deferred out-DMAs only now, AFTER this bank's exp,
                # and keep two banks in flight: by then the epilogue DVE ops
                # they wait on have finished, so they can't head-of-line
                # block the next exp on the ACT ring
                while len(pending_out) > 2:
                    pending_out.pop(0)()
                if kind == "l0":
                    for jl in range(4):
                        chunk = 4 * j + jl
                        t, coff = CHUNK_TILE[chunk]
                        _, vt = kv(t)
                        nc.tensor.matmul(
                            out=l0acc[:],
                            lhsT=et[:, 128 * jl:128 * jl + 128],
                            rhs=vt[:, coff * VW:coff * VW + VW],
                            start=(chunk == 0), stop=(chunk == L0_CH - 1),
                        )
                    if 4 * j + 3 == L0_CH - 1:
                        # bounce the L0 partial through DRAM into the
                        # per-(g) layout (ACT ring: sync ring stays pure K/V)
                        nc.vector.tensor_copy(l0sb[:], l0acc[:])
                        nc.scalar.dma_start(
                            l0b_dram[0:128 * VW], l0sb[:])
                        nc.scalar.dma_start(
                            l0ss[:],
                            l0b_dram[0:128 * VW].rearrange(
                                "(s p w) -> p s w", p=4, w=VW),
                        )
                else:
                    outb = obpool.tile([4, 4 * D], F32, tag="outb")
                    for bl in range(n):
                        s = j + bl
                        sa = sapool.tile([4, VW], F32, tag="sa")
                        for c in range(SEQ_CH):
                            chunk = L0_CH + s * SEQ_CH + c
                            t, coff = CHUNK_TILE[chunk]
                            _, vt = kv(t)
                            nc.tensor.matmul(
                                out=sa[:],
                                lhsT=et[:, 48 * bl + 4 * c:48 * bl + 4 * c + 4],
                                rhs=vt[:, coff * VW:coff * VW + VW],
                                start=(c == 0), stop=(c == SEQ_CH - 1),
                            )
                        # merge with the shared-L0 partial and divide; all
                        # operands at partition base 0
                        ss = sspool.tile([4, VW], F32, tag="ss")
                        nc.vector.tensor_add(
                            ss[:], sa[:], l0ss[:, s * VW:(s + 1) * VW])
                        r = rpool.tile([4, 1], F32, tag="r")
                        nc.vector.reciprocal(r[:], ss[:, D:D + 1])
                        nc.vector.tensor_scalar_mul(
                            outb[:, bl * D:(bl + 1) * D], ss[:, 0:D], r[:])
                    # one DMA writes this bank's output rows (ACT ring),
                    # deferred past the next bank's exp
                    def _out(j=j, n=n, outb=outb):
                        nc.scalar.dma_start(
                            out_ext[4 * j:4 * (j + n), :].rearrange(
                                "(s p) w -> p s w", p=4),
                            outb[:, 0:n * D],
                        )
                    pending_out.append(_out)

            pending = None
            for bi, bank in enumerate(banks):
                sc = emit_scores(bank)
                if pending is not None:
                    emit_tail(*pending)
                    if bi >= len(banks) - 2:
                        # near the end: flush promptly so the final out-DMAs
                        # overlap the last banks' PV instead of the drain
                        while pending_out:
                            pending_out.pop(0)()
                pending = (bank, sc)
            emit_tail(*pending)
            while pending_out:
                pending_out.pop(0)()

    nc.compile()
    return nc


def host_prep(q, kv_cache, shared_page_idx, seq1_page_idx, seq2_page_idx, np_dt):
    """Builds the 8 per-core input maps."""
    q = np.asarray(q, dtype=np.float32)
    kv = np.asarray(kv_cache, dtype=np.float32)
    sp = np.asarray(shared_page_idx).astype(np.int64).reshape(-1)
    s1 = np.asarray(seq1_page_idx).astype(np.int64)
    s2 = np.asarray(seq2_page_idx).astype(np.int64)

    per_seq = np.concatenate([s1, s2], axis=1).reshape(-1)       # [B*96]
    order = np.concatenate([sp, per_seq])                        # [3328]
    g = kv[order]                                                # [3328, 2, 16, 8, 128]
    gk = g[:, 0].reshape(T_ALL, HKV, D)
    gv = g[:, 1].reshape(T_ALL, HKV, D)

    q4 = q.reshape(B, HKV, G, D)
    in_maps = []
    for k in range(HKV):
        kh = np.ascontiguousarray(gk[:, k, :].T).astype(np_dt)   # [128, T_ALL]
        va = np.empty((T_ALL, VW), dtype=np.float32)
        va[:, :D] = gv[:, k, :]
        va[:, D] = 1.0
        vh = np.ascontiguousarray(
            va.reshape(CH, 128, VW).transpose(1, 0, 2)
        ).reshape(128, CH * VW).astype(np_dt)
        qh = np.ascontiguousarray(
            q4[:, k].transpose(2, 0, 1)
        ).reshape(D, B * G).astype(np_dt)                        # [128 d, (b,g)]
        in_maps.append({"k": kh, "v": vh, "qt": qh})
    return in_maps


def assemble_out(results):
    outs = [np.asarray(results[k]["out"]).reshape(B, G, D) for k in range(HKV)]
    return np.ascontiguousarray(
        np.stack(outs, axis=1).reshape(B, HKV * G * D)
    ).astype(np.float32)


_NC_CACHE = {}


def get_nc():
    dt, np_dt = _dtype_cfg()
    key = str(dt)
    if key not in _NC_CACHE:
        _NC_CACHE[key] = build_nc(dt)
    return _NC_CACHE[key], np_dt


def kernel(q, kv_cache, shared_page_idx, seq1_page_idx, seq2_page_idx):
    nc, np_dt = get_nc()
    in_maps = host_prep(
        q, kv_cache, shared_page_idx, seq1_page_idx, seq2_page_idx, np_dt
    )
    trace = bool(int(os.environ.get("KERNEL_TRACE", "0")))
    res = run_bass_kernel_spmd(
        nc, in_maps, core_ids=list(range(HKV)), trace=trace,
    )
    if trace and res.exec_time_ns is not None:
        print(f"HW exec time: {res.exec_time_ns} ns")
        kernel.last_exec_time_ns = res.exec_time_ns
    kernel.last_results = res
    return assemble_out(res.results)



# revision 7
# speedup vs baseline: 1.3145x; 1.3145x over previous
"""Cascade (multi-level paged) attention, distributed over 8 TRN2 NeuronCores.

Sharding: tensor-parallel over the 8 KV heads — core k owns kv-head k and its
4 GQA query heads for all 32 sequences.  Each core then reads exactly 1/8 of
the paged KV cache (its head's slice of the shared L0 prefix plus every
sequence's L1/L2 pages) from HBM once, which is the minimum possible traffic,
and no inter-core communication is needed.

Host-side prep (part of kernel(), done in numpy):
  * gather pages in the order [L0 | seq0 L1,L2 | seq1 L1,L2 | ...] using the
    page-index tensors,
  * K laid out d-major  [128 d, 53248 tok]  (matmul stationary operand),
  * V laid out token-major with a ones-column appended [tok, 129] and
    pre-swizzled into [128 tok-in-chunk, 416 chunk * 129] so each 128-token
    chunk is a direct SBUF slice; the ones column makes the softmax
    denominator fall out of the PV matmul's last output column,
  * q transposed to [128 d, 128 (seq,group)] per core.

Device kernel (per core), streaming 128-token chunks:
  scores^T chunk = matmul(lhsT=K_chunk [d,128tok], rhs=qT [d,nq]) -> PSUM
  probs = exp(scale * scores) via ScalarE (no max subtraction: scores are
  ~N(0,1) after scaling, exp is safe in f32, and partial attention sums
  become directly addable so the shared-L0 partial and per-sequence partial
  merge with a single add)
  out  += matmul(lhsT=probs^T chunk [tok,nq], rhs=[V|1] chunk [tok,129])
  epilogue per seq: (seq partial + L0 partial)[:, :128] * (1/[..., 128]).

Scores for many chunks are batched into one PSUM bank so one ACT exp call
covers up to 512 columns.  Banks are software-pipelined: PV of bank i is
emitted after the score matmuls of bank i+1 so the PE never waits on ACT.
The shared-L0 banks run first; their partial is bounced once through DRAM
into a [4 (g), seq*129] layout so each seq bank merges, divides and writes
its own output rows locally (DVE ops cannot address partition offsets that
are not multiples of 32, DRAM APs can). The final seq banks shrink to 1
sequence so the tail after the last KV byte lands is ~1.5 us.
"""

import os
from contextlib import ExitStack

import numpy as np
import ml_dtypes

import concourse.mybir as mybir
import concourse.tile as tile
from concourse import bacc
from concourse.bass_utils import run_bass_kernel_spmd

# Per-seq (L1/L2) K and V are stored as fp8 E3M4 (4 mantissa bits), halving
# the dominant HBM traffic.  The PE upconverts each matmul operand
# independently, so fp8 lhsT (K) x bf16 rhs (q) and bf16 lhsT (probs) x fp8
# rhs (V) are legal mixed-dtype matmuls and need no extra passes.  The shared
# L0 prefix (7.7% of traffic) stays bf16, as do q and the probs, keeping the
# end-to-end rel err ~1.1e-2 (numpy-validated) vs the 2e-2 gate.

# ---- problem constants (hardcoded; kernel.py must be self-contained) ----
B = 32          # sequences
HKV = 8         # kv heads == number of cores
G = 4           # query heads per kv head
D = 128         # head dim
L0_T = 4096     # shared-prefix tokens
SEQ_T = 1536    # per-sequence tokens (L1 1024 + L2 512)
T_ALL = L0_T + B * SEQ_T        # 53248
CH = T_ALL // 128               # 416 chunks of 128 tokens
L0_CH = L0_T // 128             # 32
SEQ_CH = SEQ_T // 128           # 12
SCALE = 0.08838834764831845     # D ** -0.5
VW = D + 1                      # V width incl. ones column

# chunks per DMA tile. L0 (processed first) starts with a small tile so the
# PE can start early; seq-region tiles shrink at the end so the last-arriving
# data feeds only a tiny epilogue.  Each tile must lie entirely inside one
# dtype region (chunks 0..31 = L0 bf16, chunks 32.. = seq fp8).
TILE_CHUNKS = [8, 24] + [48] * 7 + [24, 12, 12]
assert sum(TILE_CHUNKS) == CH
TILE_START = [sum(TILE_CHUNKS[:i]) for i in range(len(TILE_CHUNKS))]
CHUNK_TILE = []                 # chunk -> (tile idx, chunk offset within tile)
for t, n in enumerate(TILE_CHUNKS):
    for c in range(n):
        CHUNK_TILE.append((t, c))
# seq banks: (first seq, count); sized 4 for most, shrinking at the end
SEQ_BANKS = [(0, 4), (4, 4), (8, 4), (12, 4), (16, 4), (20, 4), (24, 4),
             (28, 2), (30, 1), (31, 1)]

F32 = mybir.dt.float32
SEQ_T_ALL = B * SEQ_T           # 49152 seq-region tokens
SEQ_CH_ALL = SEQ_T_ALL // 128   # 384 seq-region chunks


def _dtype_cfg():
    # (bf16 dtype for L0/q/probs, np dtype, fp8 seq-region enabled)
    fp8 = bool(int(os.environ.get("KERNEL_FP8", "1")))
    return mybir.dt.bfloat16, ml_dtypes.bfloat16, fp8


def build_nc(dt, fp8):
    """Builds the single-core Bass/Tile graph (same graph runs SPMD on 8 cores)."""
    f8 = mybir.dt.float8e3 if fp8 else dt
    nc = bacc.Bacc("TRN2", target_bir_lowering=False, debug=False)
    k0_ext = nc.declare_dram_parameter("k0", [128, L0_T], dt, isOutput=False)
    ks_ext = nc.declare_dram_parameter("ks", [128, SEQ_T_ALL], f8, isOutput=False)
    v0_ext = nc.declare_dram_parameter("v0", [128, L0_CH * VW], dt, isOutput=False)
    vs_ext = nc.declare_dram_parameter("vs", [128, SEQ_CH_ALL * VW], f8, isOutput=False)
    q_ext = nc.declare_dram_parameter("qt", [128, B * G], dt, isOutput=False)
    out_ext = nc.declare_dram_parameter("out", [B * G, D], F32, isOutput=True)
    l0b_dram = nc.dram_tensor("bounce", [B * G * VW], F32)

    # bank schedule: a "bank" is one PSUM score tile [128, <=512].
    # L0 banks (4 chunks x 128 qcols) run first; their merged partial is
    # bounced through DRAM into a [4, B*VW] (partition=g, seq along free)
    # layout so every seq bank can merge + divide + write its own output rows
    # locally - the tail after the last DMA is one tiny epilogue.
    banks = [("l0", j, None) for j in range(L0_CH // 4)] \
        + [("seq", s0, n) for (s0, n) in SEQ_BANKS]

    kv_bufs = 5 if dt == mybir.dt.bfloat16 else 3
    with tile.TileContext(nc) as tc:
        with ExitStack() as ctx:
            kpool = ctx.enter_context(tc.tile_pool(name="kp", bufs=kv_bufs))
            vpool = ctx.enter_context(tc.tile_pool(name="vp", bufs=kv_bufs))
            qpool = ctx.enter_context(tc.tile_pool(name="qp", bufs=1))
            epool = ctx.enter_context(tc.tile_pool(name="ep", bufs=2))
            apool = ctx.enter_context(tc.tile_pool(name="ap", bufs=1))
            sspool = ctx.enter_context(tc.tile_pool(name="ssp", bufs=6))
            obpool = ctx.enter_context(tc.tile_pool(name="obp", bufs=5))
            rpool = ctx.enter_context(tc.tile_pool(name="rp", bufs=8))
            scpool = ctx.enter_context(tc.tile_pool(name="scp", bufs=3, space="PSUM"))
            l0pool = ctx.enter_context(tc.tile_pool(name="l0p", bufs=1, space="PSUM"))
            sapool = ctx.enter_context(tc.tile_pool(name="sap", bufs=4, space="PSUM"))

            qt = qpool.tile([128, B * G], dt, tag="qt")
            nc.sync.dma_start(qt[:], q_ext[:])

            l0acc = l0pool.tile([128, VW], F32, tag="l0acc")
            l0sb = apool.tile([128, VW], F32, tag="l0sb")
            # L0 partial rearranged to partition=g, seq along the free dim
            l0ss = apool.tile([4, B * VW], F32, tag="l0ss")

            ktiles, vtiles = {}, {}

            def kv(t):
                if t not in ktiles:
                    n, c0 = TILE_CHUNKS[t], TILE_START[t]
                    if c0 >= L0_CH:         # seq region (fp8)
                        tdt, kext, vext, cb = f8, ks_ext, vs_ext, c0 - L0_CH
                        assert c0 >= L0_CH
                    else:                   # shared L0 prefix (bf16)
                        tdt, kext, vext, cb = dt, k0_ext, v0_ext, c0
                        assert c0 + n <= L0_CH
                    kt = kpool.tile([128, n * 128], tdt, tag="kt")
                    vt = vpool.tile([128, n * VW], tdt, tag="vt")
                    # split big tiles into half-DMAs so the PE's wait per
                    # score group stays under the ~3.4us HAM idle window
                    h = n // 2 if n >= 48 else n
                    for a in range(0, n, h):
                        b = min(a + h, n)
                        nc.sync.dma_start(
                            kt[:, a * 128:b * 128],
                            kext[:, (cb + a) * 128:(cb + b) * 128])
                        nc.sync.dma_start(
                            vt[:, a * VW:b * VW],
                            vext[:, (cb + a) * VW:(cb + b) * VW])
                    ktiles[t], vtiles[t] = kt, vt
                return ktiles[t], vtiles[t]

            def emit_scores(bank):
                kind, j, n = bank
                sc = scpool.tile([128, 512], F32, tag="sc")
                if kind == "l0":
                    for jl in range(4):
                        chunk = 4 * j + jl
                        t, coff = CHUNK_TILE[chunk]
                        kt, _ = kv(t)
                        nc.tensor.matmul(
                            out=sc[:, 128 * jl:128 * jl + 128],
                            lhsT=kt[:, coff * 128:coff * 128 + 128],
                            rhs=qt[:, 0:128],
                            start=True, stop=True,
                        )
                else:
                    for bl in range(n):
                        s = j + bl
                        for c in range(SEQ_CH):
                            chunk = L0_CH + s * SEQ_CH + c
                            t, coff = CHUNK_TILE[chunk]
                            kt, _ = kv(t)
                            col = 48 * bl + 4 * c
                            nc.tensor.matmul(
                                out=sc[:, col:col + 4],
                                lhsT=kt[:, coff * 128:coff * 128 + 128],
                                rhs=qt[:, 4 * s:4 * s + 4],
                                start=True, stop=True,
                            )
                return sc

            pending_out = []

            def emit_tail(bank, sc):
                kind, j, n = bank
                used = 512 if kind == "l0" else 48 * n
                et = epool.tile([128, 512], dt, tag="et")
                nc.scalar.activation(
                    et[:, :used], sc[:, :used],
                    mybir.ActivationFunctionType.Exp, scale=SCALE,
                )
                # flush deferred out-DMAs only now, AFTER this bank's exp,
                # and keep two banks in flight: by then the epilogue DVE ops
                # they wait on have finished, so they can't head-of-line
                # block the next exp on the ACT ring
                while len(pending_out) > 2:
                    pending_out.pop(0)()
                if kind == "l0":
                    for jl in range(4):
                        chunk = 4 * j + jl
                        t, coff = CHUNK_TILE[chunk]
                        _, vt = kv(t)
                        nc.tensor.matmul(
                            out=l0acc[:],
                            lhsT=et[:, 128 * jl:128 * jl + 128],
                            rhs=vt[:, coff * VW:coff * VW + VW],
                            start=(chunk == 0), stop=(chunk == L0_CH - 1),
                        )
                    if 4 * j + 3 == L0_CH - 1:
                        # bounce the L0 partial through DRAM into the
                        # per-(g) layout (ACT ring: sync ring stays pure K/V)
                        nc.vector.tensor_copy(l0sb[:], l0acc[:])
                        nc.scalar.dma_start(
                            l0b_dram[0:128 * VW], l0sb[:])
                        nc.scalar.dma_start(
                            l0ss[:],
                            l0b_dram[0:128 * VW].rearrange(
                                "(s p w) -> p s w", p=4, w=VW),
                        )
                else:
                    outb = obpool.tile([4, 4 * D], F32, tag="outb")
                    for bl in range(n):
                        s = j + bl
                        sa = sapool.tile([4, VW], F32, tag="sa")
                        for c in range(SEQ_CH):
                            chunk = L0_CH + s * SEQ_CH + c
                            t, coff = CHUNK_TILE[chunk]
                            _, vt = kv(t)
                            nc.tensor.matmul(
                                out=sa[:],
                                lhsT=et[:, 48 * bl + 4 * c:48 * bl + 4 * c + 4],
                                rhs=vt[:, coff * VW:coff * VW + VW],
                                start=(c == 0), stop=(c == SEQ_CH - 1),
                            )
                        # merge with the shared-L0 partial and divide; all
                        # operands at partition base 0
                        ss = sspool.tile([4, VW], F32, tag="ss")
                        nc.vector.tensor_add(
                            ss[:], sa[:], l0ss[:, s * VW:(s + 1) * VW])
                        r = rpool.tile([4, 1], F32, tag="r")
                        nc.vector.reciprocal(r[:], ss[:, D:D + 1])
                        nc.vector.tensor_scalar_mul(
                            outb[:, bl * D:(bl + 1) * D], ss[:, 0:D], r[:])
                    # one DMA writes this bank's output rows (ACT ring),
                    # deferred past the next bank's exp
                    def _out(j=j, n=n, outb=outb):
                        nc.scalar.dma_start(
                            out_ext[4 * j:4 * (j + n), :].rearrange(
                                "(s p) w -> p s w", p=4),
                            outb[:, 0:n * D],
                        )
                    pending_out.append(_out)

            pending = None
            for bi, bank in enumerate(banks):
                sc = emit_scores(bank)
                if pending is not None:
                    emit_tail(*pending)
                    if bi >= len(banks) - 2:
                        # near the end: flush promptly so the final out-DMAs
                        # overlap the last banks' PV instead of the drain
                        while pending_out:
                            pending_out.pop(0)()
                pending = (bank, sc)
            emit_tail(*pending)
            while pending_out:
                pending_out.pop(0)()

    nc.compile()
    return nc


def host_prep(q, kv_cache, shared_page_idx, seq1_page_idx, seq2_page_idx,
              np_dt, fp8=True):
    """Builds the 8 per-core input maps."""
    np_f8 = ml_dtypes.float8_e3m4 if fp8 else np_dt
    q = np.asarray(q, dtype=np.float32)
    kv = np.asarray(kv_cache, dtype=np.float32)
    sp = np.asarray(shared_page_idx).astype(np.int64).reshape(-1)
    s1 = np.asarray(seq1_page_idx).astype(np.int64)
    s2 = np.asarray(seq2_page_idx).astype(np.int64)

    per_seq = np.concatenate([s1, s2], axis=1).reshape(-1)       # [B*96]
    order = np.concatenate([sp, per_seq])                        # [3328]
    g = kv[order]                                                # [3328, 2, 16, 8, 128]
    gk = g[:, 0].reshape(T_ALL, HKV, D)
    gv = g[:, 1].reshape(T_ALL, HKV, D)

    q4 = q.reshape(B, HKV, G, D)
    in_maps = []
    for k in range(HKV):
        kh = np.ascontiguousarray(gk[:, k, :].T)                 # [128, T_ALL] f32
        va = np.empty((T_ALL, VW), dtype=np.float32)
        va[:, :D] = gv[:, k, :]
        va[:, D] = 1.0
        vh = np.ascontiguousarray(
            va.reshape(CH, 128, VW).transpose(1, 0, 2)
        ).reshape(128, CH * VW)                                  # [128, CH*VW] f32
        qh = np.ascontiguousarray(
            q4[:, k].transpose(2, 0, 1)
        ).reshape(D, B * G).astype(np_dt)                        # [128 d, (b,g)]
        in_maps.append({
            "k0": kh[:, :L0_T].astype(np_dt),
            "ks": kh[:, L0_T:].astype(np_f8),
            "v0": vh[:, :L0_CH * VW].astype(np_dt),
            "vs": vh[:, L0_CH * VW:].astype(np_f8),
            "qt": qh,
        })
    return in_maps


def assemble_out(results):
    outs = [np.asarray(results[k]["out"]).reshape(B, G, D) for k in range(HKV)]
    return np.ascontiguousarray(
        np.stack(outs, axis=1).reshape(B, HKV * G * D)
    ).astype(np.float32)


_NC_CACHE = {}


def get_nc():
    dt, np_dt, fp8 = _dtype_cfg()
    key = (str(dt), fp8)
    if key not in _NC_CACHE:
        _NC_CACHE[key] = build_nc(dt, fp8)
    return _NC_CACHE[key], np_dt, fp8


def kernel(q, kv_cache, shared_page_idx, seq1_page_idx, seq2_page_idx):
    nc, np_dt, fp8 = get_nc()
    in_maps = host_prep(
        q, kv_cache, shared_page_idx, seq1_page_idx, seq2_page_idx, np_dt, fp8
    )
    trace = bool(int(os.environ.get("KERNEL_TRACE", "0")))
    res = run_bass_kernel_spmd(
        nc, in_maps, core_ids=list(range(HKV)), trace=trace,
    )
    if trace and res.exec_time_ns is not None:
        print(f"HW exec time: {res.exec_time_ns} ns")
        kernel.last_exec_time_ns = res.exec_time_ns
    kernel.last_results = res
    return assemble_out(res.results)



# revision 16
# speedup vs baseline: 1.3654x; 1.0387x over previous
"""Cascade (multi-level paged) attention, distributed over 8 TRN2 NeuronCores.

Sharding: tensor-parallel over the 8 KV heads — core k owns kv-head k and its
4 GQA query heads for all 32 sequences.  Each core then reads exactly 1/8 of
the paged KV cache (its head's slice of the shared L0 prefix plus every
sequence's L1/L2 pages) from HBM once, which is the minimum possible traffic,
and no inter-core communication is needed.

Traffic: the per-seq (L1/L2) K and V — 92% of bytes — are stored as fp8 E3M4
(4 mantissa bits), halving the dominant HBM traffic.  The PE upconverts each
matmul operand independently, so fp8 lhsT (K) x bf16 rhs (q) and fp8 lhsT (V)
x bf16 rhs (probs) are legal mixed-dtype matmuls and need no extra passes.
The shared L0 prefix stays bf16, as do q and the probs, keeping end-to-end
rel err ~1.2e-2 (numpy+CoreSim validated) vs the 2e-2 gate.

Host-side prep (part of kernel(), done in numpy):
  * gather pages in the order [L0 | seq0 L1,L2 | seq1 L1,L2 | ...] using the
    page-index tensors,
  * K laid out d-major [128 d, tok] (matmul stationary operand for scores),
  * V laid out token-major per 128-chunk [128 tok-in-chunk, chunk * 128 d]
    (matmul stationary operand for PV),
  * q transposed to [128 d, 128 (seq,group)] per core,
  * aux constants: 128x128 identity, ones column (for denominators).

Device kernel (per core), streaming 128-token chunks:
  scores^T chunk = matmul(lhsT=K_chunk [d,128tok], rhs=qT [d,nq]) -> PSUM
  probs = exp(scale * scores) via ScalarE (no max subtraction: scores are
  ~N(0,1) after scaling, exp is safe, and partial attention sums become
  directly addable so the shared-L0 partial and per-sequence partial merge
  with a single add)
  denom: one matmul(lhsT=ones [tok,1], rhs=probs bank) -> per-column sums,
  then a strided DVE reduce folds the 12 chunk-groups of each sequence
  out += matmul(lhsT=V_chunk [tok,128d], rhs=probs chunk [tok,nq])
    -> PV accumulates in a [128 d, nq] PSUM: V is STATIONARY and the probs
    are the 4-column moving operand, so the PE streams 4 cols/chunk instead
    of 129 — the PE stops pacing the kernel (~28us busy vs 43us DMA).
  epilogue per bank: merge with L0 partial ([d, (s,g)] layouts now match -> a
  plain DVE add, no DRAM bounce), transpose numerator and denominator back to
  [(s,g), d] via identity matmuls, reciprocal-scale, one contiguous out-DMA.

Scores for many chunks are batched into one PSUM bank so one ACT exp call
covers up to 512 columns.  Banks are software-pipelined: the tail of bank i
is emitted after the score matmuls of bank i+1 so the PE never waits on ACT.
The final seq banks shrink to 1 sequence so the tail after the last KV byte
lands is small.
"""

import os
from contextlib import ExitStack

import numpy as np
import ml_dtypes

import concourse.mybir as mybir
import concourse.tile as tile
from concourse import bacc
from concourse.bass_utils import run_bass_kernel_spmd

# ---- problem constants (hardcoded; kernel.py must be self-contained) ----
B = 32          # sequences
HKV = 8         # kv heads == number of cores
G = 4           # query heads per kv head
D = 128         # head dim
L0_T = 4096     # shared-prefix tokens
SEQ_T = 1536    # per-sequence tokens (L1 1024 + L2 512)
T_ALL = L0_T + B * SEQ_T        # 53248
CH = T_ALL // 128               # 416 chunks of 128 tokens
L0_CH = L0_T // 128             # 32
SEQ_CH = SEQ_T // 128           # 12
SCALE = 0.08838834764831845     # D ** -0.5
SEQ_T_ALL = B * SEQ_T           # 49152 seq-region tokens
SEQ_CH_ALL = SEQ_T_ALL // 128   # 384 seq-region chunks

# chunks per DMA tile. L0 (processed first) starts with a small tile so the
# PE can start early; seq-region tiles shrink at the end so the last-arriving
# data feeds only a tiny epilogue.  Each tile must lie entirely inside one
# dtype region (chunks 0..31 = L0 bf16, chunks 32.. = seq fp8).
TILE_CHUNKS = [4, 4, 24] + [48] * 7 + [24, 12, 12]
assert sum(TILE_CHUNKS) == CH
TILE_START = [sum(TILE_CHUNKS[:i]) for i in range(len(TILE_CHUNKS))]
CHUNK_TILE = []                 # chunk -> (tile idx, chunk offset within tile)
for t, n in enumerate(TILE_CHUNKS):
    for c in range(n):
        CHUNK_TILE.append((t, c))
# seq banks: (first seq, count); sized 4 for most, shrinking at the end
SEQ_BANKS = [(0, 4), (4, 4), (8, 4), (12, 4), (16, 4), (20, 4), (24, 4),
             (28, 2), (30, 1), (31, 1)]

F32 = mybir.dt.float32


def _dtype_cfg():
    fp8 = bool(int(os.environ.get("KERNEL_FP8", "1")))
    return mybir.dt.bfloat16, ml_dtypes.bfloat16, fp8


def build_nc(dt, fp8):
    """Builds the single-core Bass/Tile graph (same graph runs SPMD on 8 cores)."""
    f8 = mybir.dt.float8e3 if fp8 else dt
    nc = bacc.Bacc("TRN2", target_bir_lowering=False, debug=False)
    k0_ext = nc.declare_dram_parameter("k0", [128, L0_T], dt, isOutput=False)
    ks_ext = nc.declare_dram_parameter("ks", [128, SEQ_T_ALL], f8, isOutput=False)
    v0_ext = nc.declare_dram_parameter("v0", [128, L0_T], dt, isOutput=False)
    vs_ext = nc.declare_dram_parameter("vs", [128, SEQ_T_ALL], f8, isOutput=False)
    q_ext = nc.declare_dram_parameter("qt", [128, B * G], dt, isOutput=False)
    id_ext = nc.declare_dram_parameter("ident", [128, 128], dt, isOutput=False)
    on_ext = nc.declare_dram_parameter("onesb", [128, 1], dt, isOutput=False)
    o1_ext = nc.declare_dram_parameter("one1", [1, 1], dt, isOutput=False)
    out_ext = nc.declare_dram_parameter("out", [B * G, D], F32, isOutput=True)

    # bank schedule: a "bank" is one PSUM score tile [128, <=512].
    # L0 banks (4 chunks x 128 qcols) run first and accumulate the shared
    # partial into l0pv [128 d, 128 (s,g)] / l0den [1, 128]; every seq bank
    # then merges + divides + writes its own output rows locally.
    banks = [("l0", j, None) for j in range(L0_CH // 4)] \
        + [("seq", s0, n) for (s0, n) in SEQ_BANKS]

    with tile.TileContext(nc) as tc:
        with ExitStack() as ctx:
            kpool = ctx.enter_context(tc.tile_pool(name="kp", bufs=6))
            vpool = ctx.enter_context(tc.tile_pool(name="vp", bufs=6))
            qpool = ctx.enter_context(tc.tile_pool(name="qp", bufs=1))
            epool = ctx.enter_context(tc.tile_pool(name="ep", bufs=2))
            mpool = ctx.enter_context(tc.tile_pool(name="mp", bufs=2))
            dpool = ctx.enter_context(tc.tile_pool(name="dp", bufs=2))
            rpool = ctx.enter_context(tc.tile_pool(name="rp", bufs=4))
            opool = ctx.enter_context(tc.tile_pool(name="op", bufs=3))
            scpool = ctx.enter_context(tc.tile_pool(name="scp", bufs=3, space="PSUM"))
            l0pool = ctx.enter_context(tc.tile_pool(name="l0p", bufs=1, space="PSUM"))
            # one PSUM bank per in-flight seq-bank tail, manually packed:
            # cols [0:16) sa PV-accum, [16:208) den_b, [208:336) tr, [336:337) dtp
            tlpool = ctx.enter_context(tc.tile_pool(name="tlp", bufs=2, space="PSUM"))

            qt = qpool.tile([128, B * G], dt, tag="qt")
            ident = qpool.tile([128, 128], dt, tag="ident")
            onesb = qpool.tile([128, 1], dt, tag="onesb")
            one1 = qpool.tile([1, 1], dt, tag="one1")
            nc.scalar.dma_start(qt[:], q_ext[:])
            nc.scalar.dma_start(ident[:], id_ext[:])
            nc.scalar.dma_start(onesb[:], on_ext[:])
            nc.scalar.dma_start(one1[:], o1_ext[:])

            l0pv = l0pool.tile([128, 128], F32, tag="l0pv")   # [128 d, (s,g)]
            l0den = l0pool.tile([1, 128], F32, tag="l0den")   # [1, (s,g)]
            # SBUF copy of l0pv: a DVE op may read only one PSUM operand, so
            # the per-bank merge add reads this copy instead of the PSUM bank
            l0sb = qpool.tile([128, 128], F32, tag="l0sb")

            ktiles, vtiles = {}, {}

            def kv(t):
                if t not in ktiles:
                    n, c0 = TILE_CHUNKS[t], TILE_START[t]
                    if c0 >= L0_CH:         # seq region (fp8)
                        tdt, kext, vext, cb = f8, ks_ext, vs_ext, c0 - L0_CH
                    else:                   # shared L0 prefix (bf16)
                        tdt, kext, vext, cb = dt, k0_ext, v0_ext, c0
                        assert c0 + n <= L0_CH
                    kt = kpool.tile([128, n * 128], tdt, tag="kt")
                    vt = vpool.tile([128, n * 128], tdt, tag="vt")
                    # split big tiles into half-DMAs so the PE's wait per
                    # score group stays under the ~3.4us HAM idle window
                    h = n // 2 if n >= 48 else n
                    for a in range(0, n, h):
                        b = min(a + h, n)
                        nc.sync.dma_start(
                            kt[:, a * 128:b * 128],
                            kext[:, (cb + a) * 128:(cb + b) * 128])
                        nc.sync.dma_start(
                            vt[:, a * 128:b * 128],
                            vext[:, (cb + a) * 128:(cb + b) * 128])
                    ktiles[t], vtiles[t] = kt, vt
                return ktiles[t], vtiles[t]

            def emit_scores(bank):
                kind, j, n = bank
                sc = scpool.tile([128, 512], F32, tag="sc")
                if kind == "l0":
                    for jl in range(4):
                        chunk = 4 * j + jl
                        t, coff = CHUNK_TILE[chunk]
                        kt, _ = kv(t)
                        nc.tensor.matmul(
                            out=sc[:, 128 * jl:128 * jl + 128],
                            lhsT=kt[:, coff * 128:coff * 128 + 128],
                            rhs=qt[:, 0:128],
                            start=True, stop=True,
                        )
                else:
                    for bl in range(n):
                        s = j + bl
                        for c in range(SEQ_CH):
                            chunk = L0_CH + s * SEQ_CH + c
                            t, coff = CHUNK_TILE[chunk]
                            kt, _ = kv(t)
                            col = 48 * bl + 4 * c
                            nc.tensor.matmul(
                                out=sc[:, col:col + 4],
                                lhsT=kt[:, coff * 128:coff * 128 + 128],
                                rhs=qt[:, 4 * s:4 * s + 4],
                                start=True, stop=True,
                            )
                return sc

            pending_out = []

            def emit_tail(bank, sc):
                kind, j, n = bank
                used = 512 if kind == "l0" else 48 * n
                et = epool.tile([128, 512], dt, tag="et")
                nc.scalar.activation(
                    et[:, :used], sc[:, :used],
                    mybir.ActivationFunctionType.Exp, scale=SCALE,
                )
                # flush deferred out-DMAs only now, AFTER this bank's exp,
                # and keep two banks in flight: by then the epilogue DVE ops
                # they wait on have finished, so they can't head-of-line
                # block the next exp on the ACT ring
                while len(pending_out) > 2:
                    pending_out.pop(0)()
                if kind == "l0":
                    # denominator column-sums for this bank's 4 chunks
                    for jl in range(4):
                        chunk = 4 * j + jl
                        nc.tensor.matmul(
                            out=l0den[:],
                            lhsT=onesb[:],
                            rhs=et[:, 128 * jl:128 * jl + 128],
                            start=(chunk == 0), stop=(chunk == L0_CH - 1),
                        )
                    # PV: V stationary, probs moving
                    for jl in range(4):
                        chunk = 4 * j + jl
                        t, coff = CHUNK_TILE[chunk]
                        _, vt = kv(t)
                        nc.tensor.matmul(
                            out=l0pv[:],
                            lhsT=vt[:, coff * 128:coff * 128 + 128],
                            rhs=et[:, 128 * jl:128 * jl + 128],
                            start=(chunk == 0), stop=(chunk == L0_CH - 1),
                        )
                    if 4 * j + 3 == L0_CH - 1:
                        nc.vector.tensor_copy(l0sb[:], l0pv[:])
                else:
                    nq = 4 * n
                    tb = tlpool.tile([128, 512], F32, tag="tb")
                    sa = tb[:, 0:16]
                    den_b = tb[0:1, 16:208]
                    tr = tb[0:16, 208:336]
                    dtp = tb[0:16, 336:337]
                    # per-column sums of the whole probs bank in one matmul
                    nc.tensor.matmul(
                        out=den_b[:, :used], lhsT=onesb[:], rhs=et[:, :used],
                        start=True, stop=True,
                    )
                    # PV accumulation: [128 d, 4] per seq, V stationary
                    for bl in range(n):
                        s = j + bl
                        for c in range(SEQ_CH):
                            chunk = L0_CH + s * SEQ_CH + c
                            t, coff = CHUNK_TILE[chunk]
                            _, vt = kv(t)
                            nc.tensor.matmul(
                                out=sa[:, 4 * bl:4 * bl + 4],
                                lhsT=vt[:, coff * 128:coff * 128 + 128],
                                rhs=et[:, 48 * bl + 4 * c:48 * bl + 4 * c + 4],
                                start=(c == 0), stop=(c == SEQ_CH - 1),
                            )
                    # fold the 12 chunk-groups of each seq: [1,(n c g)] ->
                    # view [1,n,g,c] and reduce innermost
                    dred = rpool.tile([1, 16], F32, tag="dred")
                    nc.vector.tensor_reduce(
                        dred[:, :nq],
                        den_b[:, :used].rearrange(
                            "p (n c g) -> p n g c", c=SEQ_CH, g=4),
                        axis=mybir.AxisListType.X, op=mybir.AluOpType.add,
                    )
                    # merge with shared-L0 partials (same layouts, plain adds)
                    dent = rpool.tile([1, 16], dt, tag="dent")
                    nc.vector.tensor_add(
                        dent[:, :nq], dred[:, :nq], l0den[:, 4 * j:4 * j + nq])
                    msb = mpool.tile([128, 16], dt, tag="msb")
                    nc.vector.tensor_add(
                        msb[:, :nq], sa[:, :nq], l0sb[:, 4 * j:4 * j + nq])
                    # transpose numerator [128 d, nq] -> [nq, 128 d] and
                    # denominator [1, nq] -> [nq, 1] via identity matmuls
                    nc.tensor.matmul(
                        out=tr[:nq, :], lhsT=msb[:, :nq], rhs=ident[:],
                        start=True, stop=True,
                    )
                    nc.tensor.matmul(
                        out=dtp[:nq, :], lhsT=dent[:, :nq], rhs=one1[:],
                        start=True, stop=True,
                    )
                    r = rpool.tile([16, 1], F32, tag="r")
                    nc.vector.reciprocal(r[:nq], dtp[:nq, :])
                    ob = opool.tile([16, 128], F32, tag="ob")
                    nc.vector.tensor_scalar_mul(ob[:nq, :], tr[:nq, :], r[:nq])

                    # one contiguous DMA writes this bank's output rows,
                    # deferred past the next bank's exp
                    def _out(j=j, nq=nq, ob=ob):
                        nc.scalar.dma_start(
                            out_ext[4 * j:4 * j + nq, :], ob[:nq, :])
                    pending_out.append(_out)

            pending = None
            for bi, bank in enumerate(banks):
                sc = emit_scores(bank)
                if pending is not None:
                    emit_tail(*pending)
                    if bi >= len(banks) - 2:
                        # near the end: flush promptly so the final out-DMAs
                        # overlap the last banks' PV instead of the drain
                        while pending_out:
                            pending_out.pop(0)()
                pending = (bank, sc)
            emit_tail(*pending)
            while pending_out:
                pending_out.pop(0)()

    nc.compile()
    return nc


def host_prep(q, kv_cache, shared_page_idx, seq1_page_idx, seq2_page_idx,
              np_dt, fp8=True):
    """Builds the 8 per-core input maps."""
    np_f8 = ml_dtypes.float8_e3m4 if fp8 else np_dt
    q = np.asarray(q, dtype=np.float32)
    kv = np.asarray(kv_cache, dtype=np.float32)
    sp = np.asarray(shared_page_idx).astype(np.int64).reshape(-1)
    s1 = np.asarray(seq1_page_idx).astype(np.int64)
    s2 = np.asarray(seq2_page_idx).astype(np.int64)

    per_seq = np.concatenate([s1, s2], axis=1).reshape(-1)       # [B*96]
    order = np.concatenate([sp, per_seq])                        # [3328]
    g = kv[order]                                                # [3328, 2, 16, 8, 128]
    gk = g[:, 0].reshape(T_ALL, HKV, D)
    gv = g[:, 1].reshape(T_ALL, HKV, D)

    q4 = q.reshape(B, HKV, G, D)
    ident = np.eye(128, dtype=np.float32).astype(np_dt)
    onesb = np.ones((128, 1), dtype=np.float32).astype(np_dt)
    one1 = np.ones((1, 1), dtype=np.float32).astype(np_dt)
    in_maps = []
    for k in range(HKV):
        kh = np.ascontiguousarray(gk[:, k, :].T)                 # [128 d, T_ALL]
        vh = np.ascontiguousarray(
            gv[:, k, :].reshape(CH, 128, D).transpose(1, 0, 2)
        ).reshape(128, CH * D)                                   # [128 tok, (c d)]
        qh = np.ascontiguousarray(
            q4[:, k].transpose(2, 0, 1)
        ).reshape(D, B * G).astype(np_dt)                        # [128 d, (b,g)]
        in_maps.append({
            "k0": kh[:, :L0_T].astype(np_dt),
            "ks": kh[:, L0_T:].astype(np_f8),
            "v0": vh[:, :L0_T].astype(np_dt),
            "vs": vh[:, L0_T:].astype(np_f8),
            "qt": qh,
            "ident": ident,
            "onesb": onesb,
            "one1": one1,
        })
    return in_maps


def assemble_out(results):
    outs = [np.asarray(results[k]["out"]).reshape(B, G, D) for k in range(HKV)]
    return np.ascontiguousarray(
        np.stack(outs, axis=1).reshape(B, HKV * G * D)
    ).astype(np.float32)


_NC_CACHE = {}


def get_nc():
    dt, np_dt, fp8 = _dtype_cfg()
    key = (str(dt), fp8)
    if key not in _NC_CACHE:
        _NC_CACHE[key] = build_nc(dt, fp8)
    return _NC_CACHE[key], np_dt, fp8


def kernel(q, kv_cache, shared_page_idx, seq1_page_idx, seq2_page_idx):
    nc, np_dt, fp8 = get_nc()
    in_maps = host_prep(
        q, kv_cache, shared_page_idx, seq1_page_idx, seq2_page_idx, np_dt, fp8
    )
    trace = bool(int(os.environ.get("KERNEL_TRACE", "0")))
    res = run_bass_kernel_spmd(
        nc, in_maps, core_ids=list(range(HKV)), trace=trace,
    )
    if trace and res.exec_time_ns is not None:
        print(f"HW exec time: {res.exec_time_ns} ns")
        kernel.last_exec_time_ns = res.exec_time_ns
    kernel.last_results = res
    return assemble_out(res.results)
